# revision 8
# baseline (speedup 1.0000x reference)
"""Two-layer GCN (GCNConv 256->128->64, leaky_relu) on 8 Trainium2 NeuronCores.

Strategy (graph/data parallel per the sharding hint):
  - Nodes are sharded contiguously across the 8 cores (12500 each); each core
    owns the destination rows of the scatter for its node range.
  - Symmetric normalization is folded into node features: with
    h~ = dinv ! (X @ W), the aggregation for node v is
    out[v] = dinv[v] * (sum_{e: dst=v} h~[src_e] + h~[v]) + b.
  - Each layer: dense transform on local nodes -> AllGather h~ (the halo
    exchange; with 8 cores and uniform random edges every core needs nearly
    every node) -> per 128-node dst block, dma_gather the source rows and
    segment-sum them with 0/1 selection-matrix matmuls accumulated in PSUM.
  - Edge tables (gather indices, dst slots) are precomputed host-side from
    edge_index; edges are sorted by (dst block group, src 32k-chunk, dst
    block) and padded to 128-edge tiles.  dma_gather indices are int16, which
    limits each gather to a 32768-row window of the source table - hence the
    4 source chunks.
"""
import math
import sys

import numpy as np

sys.path.insert(0, "/opt/trn_rl_repo")

import concourse.bacc as bacc  # noqa: E402
import concourse.mybir as mybir  # noqa: E402
import concourse.tile as tile  # noqa: E402
from concourse.bass_utils import run_bass_kernel_spmd  # noqa: E402

P = 128
F32 = mybir.dt.float32
I16 = mybir.dt.int16


class Cfg:
    def __init__(self, n, e, d0, d1, d2, ncores=8, grp=6, chunk=32768,
                 neg_slope=0.01):
        assert n % ncores == 0
        self.n, self.e = n, e
        self.d0, self.d1, self.d2 = d0, d1, d2
        self.ncores = ncores
        self.nloc = n // ncores
        self.nblk = math.ceil(self.nloc / P)
        self.grp = grp
        self.ngrp = math.ceil(self.nblk / grp)
        self.chunk = chunk
        self.nchunk = math.ceil(n / chunk)
        self.neg_slope = neg_slope

    def rows(self, b):
        return min(P, self.nloc - b * P)


FULL = Cfg(n=100000, e=1600000, d0=256, d1=128, d2=64)


# --------------------------------------------------------------------------
# host-side preprocessing
# --------------------------------------------------------------------------

def prepare(cfg, x, edge_index, W1, b1, W2, b2):
    """Shard + build per-core tables and the shared tile structure."""
    src = np.asarray(edge_index[0], dtype=np.int64)
    dst = np.asarray(edge_index[1], dtype=np.int64)

    deg = (np.bincount(dst, minlength=cfg.n) + 1).astype(np.float32)
    dinv = (np.float32(1.0) / np.sqrt(deg)).astype(np.float32)

    core = dst // cfg.nloc
    dstl = dst - core * cfg.nloc
    b = dstl >> 7
    k = src // cfg.chunk
    g = b // cfg.grp
    # stream order: core -> g -> k -> b (tiles within segment arbitrary)
    key = ((core * cfg.ngrp + g) * cfg.nchunk + k) * cfg.nblk + b
    order = np.argsort(key, kind="stable")
    src_s, b_s, k_s, core_s, dstl_s = (
        src[order], b[order], k[order], core[order], dstl[order])

    seg_id = ((core_s * cfg.nblk + b_s) * cfg.nchunk + k_s)
    counts = np.bincount(seg_id, minlength=cfg.ncores * cfg.nblk * cfg.nchunk)
    counts = counts.reshape(cfg.ncores, cfg.nblk, cfg.nchunk)
    # shared tile structure: max over cores
    T = np.ceil(counts.max(axis=0) / P).astype(np.int64)  # [nblk, nchunk]

    # segment stream offsets (in padded edge slots), shared across cores
    seg_off = np.zeros((cfg.nblk, cfg.nchunk), np.int64)
    blocks_of_g = [list(range(gg * cfg.grp, min((gg + 1) * cfg.grp, cfg.nblk)))
                   for gg in range(cfg.ngrp)]
    tot_tiles = 0
    for gg in range(cfg.ngrp):
        for kk in range(cfg.nchunk):
            for bb in blocks_of_g[gg]:
                seg_off[bb, kk] = tot_tiles * P
                tot_tiles += int(T[bb, kk])
    etot = tot_tiles * P

    # per-core padded arrays
    xs = np.asarray(x, dtype=np.float32)
    W1 = np.asarray(W1, np.float32)
    W2 = np.asarray(W2, np.float32)
    b1 = np.asarray(b1, np.float32)
    b2 = np.asarray(b2, np.float32)
    b1bc = np.tile(b1[None, :], (P, 1)).astype(np.float32)
    b2bc = np.tile(b2[None, :], (P, 1)).astype(np.float32)

    in_maps = []
    for c in range(cfg.ncores):
        m = core_s == c
        src_c, b_c, k_c, dstl_c = src_s[m], b_s[m], k_s[m], dstl_s[m]
        seg_c = (b_c * cfg.nchunk + k_c)
        # rank within segment: edges are segment-contiguous in stream order
        ne = len(src_c)
        if ne:
            newseg = np.r_[True, seg_c[1:] != seg_c[:-1]]
            seg_start = np.maximum.accumulate(
                np.where(newseg, np.arange(ne), 0))
            pos_in_seg = np.arange(ne) - seg_start
        else:
            pos_in_seg = np.zeros(0, np.int64)
        padded_pos = seg_off[b_c, k_c] + pos_in_seg

        idx_pad = np.zeros(etot, np.int16)
        idx_pad[padded_pos] = (src_c - k_c * cfg.chunk).astype(np.int16)
        dr_pad = np.full(etot, -1.0, np.float32)
        dr_pad[padded_pos] = (dstl_c - b_c * P).astype(np.float32)

        # idx table: wrapped [16, etot//16] (entry [i%16, i//16]), x8 down 128
        idx16 = idx_pad.reshape(etot // 16, 16).T.copy()
        idx_tab = np.tile(idx16, (8, 1))
        # dstrel table: [128, tot_tiles], [p, t] = edge t*128+p
        dstrel_tab = dr_pad.reshape(tot_tiles, P).T.copy()

        lo = c * cfg.nloc
        xT = np.ascontiguousarray(xs[lo:lo + cfg.nloc].T)
        dloc = dinv[lo:lo + cfg.nloc]
        dcols = np.ones(cfg.nblk * P, np.float32)
        dcols[:cfg.nloc] = dloc
        dinv_cols = dcols.reshape(cfg.nblk, P).T.copy()

        in_maps.append({
            "xT": xT, "dinv_cols": dinv_cols,
            "W1": W1, "W2": W2, "b1bc": b1bc, "b2bc": b2bc,
            "idx_tab": idx_tab, "dstrel_tab": dstrel_tab,
        })

    struct = dict(T=T, blocks_of_g=blocks_of_g, tot_tiles=tot_tiles)
    return in_maps, struct


# --------------------------------------------------------------------------
# device program
# --------------------------------------------------------------------------

def build_program(cfg, struct):
    T = struct["T"]
    blocks_of_g = struct["blocks_of_g"]
    tot_tiles = struct["tot_tiles"]
    NB, NK, NG = cfg.nblk, cfg.nchunk, cfg.ngrp
    D0, D1, D2 = cfg.d0, cfg.d1, cfg.d2
    K0 = D0 // P
    G = cfg.grp

    # per-group layout in the tile stream
    grp_tiles = []           # tiles per group
    gk_info = []             # [g][k] -> (col0 within group, ntiles)
    blk_tile_cols = {}       # b -> [(col within group, ntiles, k)]
    grp_col0 = []            # group offset in global tile stream
    tot = 0
    for g in range(NG):
        grp_col0.append(tot)
        col = 0
        ks = []
        for k in range(NK):
            c0 = col
            for b in blocks_of_g[g]:
                t = int(T[b, k])
                if t:
                    blk_tile_cols.setdefault(b, []).append((col, t, k))
                    col += t
            ks.append((c0, col - c0))
        gk_info.append(ks)
        grp_tiles.append(col)
        tot += col
    assert tot == tot_tiles
    TG = max(grp_tiles)
    TBK = max(int(T.max()), 1)

    nc = bacc.Bacc("TRN2", target_bir_lowering=False, debug=False,
                   num_devices=cfg.ncores)
    xT = nc.dram_tensor("xT", [D0, cfg.nloc], F32, kind="ExternalInput")
    dinv_t = nc.dram_tensor("dinv_cols", [P, NB], F32, kind="ExternalInput")
    W1 = nc.dram_tensor("W1", [D0, D1], F32, kind="ExternalInput")
    W2 = nc.dram_tensor("W2", [D1, D2], F32, kind="ExternalInput")
    b1t = nc.dram_tensor("b1bc", [P, D1], F32, kind="ExternalInput")
    b2t = nc.dram_tensor("b2bc", [P, D2], F32, kind="ExternalInput")
    idx_t = nc.dram_tensor("idx_tab", [P, tot_tiles * 8], I16,
                           kind="ExternalInput")
    drel_t = nc.dram_tensor("dstrel_tab", [P, tot_tiles], F32,
                            kind="ExternalInput")
    out_t = nc.dram_tensor("out_loc", [cfg.nloc, D2], F32,
                           kind="ExternalOutput")

    eq = mybir.AluOpType.is_equal
    mul = mybir.AluOpType.mult
    add = mybir.AluOpType.add
    lrelu = mybir.ActivationFunctionType.Lrelu

    with tile.TileContext(nc) as tc:
        with (
            tc.tile_pool(name="const", bufs=1) as cp,
            tc.tile_pool(name="work", bufs=2) as wp,
            tc.tile_pool(name="sm", bufs=3) as smp,
            tc.tile_pool(name="mm", bufs=4, space="PSUM") as mmp,
            tc.tile_pool(name="agg", bufs=2, space="PSUM") as aggp,
            tc.tile_pool(name="dram", bufs=1, space="DRAM") as drp,
        ):
            h1_loc = drp.tile([cfg.nloc, D1], F32, tag="h1_loc")
            h1_full = drp.tile([cfg.n, D1], F32, tag="h1_full", addr_space="Shared")
            h2_loc = drp.tile([cfg.nloc, D2], F32, tag="h2_loc")
            h2_full = drp.tile([cfg.n, D2], F32, tag="h2_full", addr_space="Shared")

            w1sb = cp.tile([P, K0, D1], F32)
            for k0 in range(K0):
                nc.sync.dma_start(out=w1sb[:, k0, :],
                                  in_=W1[k0 * P:(k0 + 1) * P, :])
            w2sb = cp.tile([P, D2], F32)
            nc.sync.dma_start(out=w2sb[:], in_=W2[:])
            b1sb = cp.tile([P, D1], F32)
            nc.sync.dma_start(out=b1sb[:], in_=b1t[:])
            b2sb = cp.tile([P, D2], F32)
            nc.sync.dma_start(out=b2sb[:], in_=b2t[:])
            dvsb = cp.tile([P, NB], F32)
            nc.sync.dma_start(out=dvsb[:], in_=dinv_t[:])
            iota = cp.tile([P, P], F32)
            nc.gpsimd.iota(iota[:], pattern=[[1, P]], base=0,
                           channel_multiplier=0,
                           allow_small_or_imprecise_dtypes=True)
            ident = cp.tile([P, P], F32)
            from concourse.masks import make_identity
            make_identity(nc, ident[:])

            # ---------------- dense layer 1: h1_loc = dinv * (x @ W1)
            for b in range(NB):
                nb = cfg.rows(b)
                xt = wp.tile([P, K0, P], F32, tag="xt")
                for k0 in range(K0):
                    nc.sync.dma_start(
                        out=xt[:, k0, :nb],
                        in_=xT[k0 * P:(k0 + 1) * P, b * P:b * P + nb])
                ps = mmp.tile([P, D1], F32, tag="mm")
                for k0 in range(K0):
                    nc.tensor.matmul(out=ps[:nb, :], lhsT=xt[:, k0, :nb],
                                     rhs=w1sb[:, k0, :],
                                     start=(k0 == 0), stop=(k0 == K0 - 1))
                h1w = wp.tile([P, D1], F32, tag="h1w")
                nc.vector.tensor_scalar(out=h1w[:nb, :], in0=ps[:nb, :],
                                        scalar1=dvsb[:nb, b:b + 1],
                                        scalar2=None, op0=mul)
                nc.sync.dma_start(out=h1_loc[b * P:b * P + nb, :],
                                  in_=h1w[:nb, :])

            nc.gpsimd.collective_compute(
                "AllGather", mybir.AluOpType.bypass,
                replica_groups=[list(range(cfg.ncores))],
                ins=[h1_loc.opt()], outs=[h1_full.opt()])

            # ---------------- layer-1 aggregation + fused layer-2 dense
            def agg_phase(layer, h_full, h_self, D, out_sink):
                for g in range(NG):
                    Tg = grp_tiles[g]
                    if Tg == 0:
                        continue
                    gt0 = grp_col0[g]
                    stage_raw = wp.tile([P, TG * D1], F32, tag="stage")
                    stage = stage_raw[:].rearrange("p (t d) -> p t d", d=D)
                    idxt = wp.tile([P, TG * 8], I16, tag="idx")
                    drt = wp.tile([P, TG], F32, tag="drel")
                    nc.sync.dma_start(out=idxt[:, :Tg * 8],
                                      in_=idx_t[:, gt0 * 8:(gt0 + Tg) * 8])
                    nc.sync.dma_start(out=drt[:, :Tg],
                                      in_=drel_t[:, gt0:gt0 + Tg])
                    for k in range(NK):
                        c0, ntk = gk_info[g][k]
                        if ntk == 0:
                            continue
                        hi = min((k + 1) * cfg.chunk, cfg.n)
                        nc.gpsimd.dma_gather(
                            out_ap=stage[:, c0:c0 + ntk, :],
                            in_ap=h_full[k * cfg.chunk:hi, :],
                            idxs_ap=idxt[:, c0 * 8:(c0 + ntk) * 8],
                            num_idxs=ntk * P, num_idxs_reg=ntk * P,
                            elem_size=D, single_packet=False)
                    agg = aggp.tile([P, G * D], F32, tag="agg")
                    for slot, b in enumerate(blocks_of_g[g]):
                        segs = blk_tile_cols.get(b, [])
                        ntot = sum(t for _, t, _ in segs)
                        nb = cfg.rows(b)
                        left = ntot
                        first = True
                        for (col, t, k) in segs:
                            sm = smp.tile([P, TBK, P], F32, tag="smulti")
                            nc.vector.tensor_tensor(
                                out=sm[:, :t, :],
                                in0=iota[:].unsqueeze(1)
                                    .broadcast_to([P, t, P]),
                                in1=drt[:, col:col + t].unsqueeze(2)
                                    .broadcast_to([P, t, P]),
                                op=eq)
                            for tt in range(t):
                                nc.tensor.matmul(
                                    out=agg[:, slot * D:(slot + 1) * D],
                                    lhsT=sm[:, tt, :],
                                    rhs=stage[:, col + tt, :],
                                    start=first, stop=(left == 1),
                                    skip_group_check=True)
                                first = False
                                left -= 1
                        # flush
                        selfb = wp.tile([P, D], F32, tag=f"selfb{layer}")
                        nc.sync.dma_start(
                            out=selfb[:nb, :],
                            in_=h_self[b * P:b * P + nb, :])
                        t0 = wp.tile([P, D], F32, tag=f"t0_{layer}")
                        if ntot:
                            nc.vector.tensor_tensor(
                                out=t0[:nb, :],
                                in0=agg[:nb, slot * D:(slot + 1) * D],
                                in1=selfb[:nb, :], op=add)
                        else:
                            nc.vector.tensor_copy(out=t0[:nb, :],
                                                  in_=selfb[:nb, :])
                        out_sink(b, nb, t0)

            def l1_sink(b, nb, t0):
                t1 = wp.tile([P, D1], F32, tag="t1")
                nc.vector.tensor_scalar(out=t1[:nb, :], in0=t0[:nb, :],
                                        scalar1=dvsb[:nb, b:b + 1],
                                        scalar2=None, op0=mul)
                t2 = wp.tile([P, D1], F32, tag="t2")
                nc.vector.tensor_tensor(out=t2[:nb, :], in0=t1[:nb, :],
                                        in1=b1sb[:nb, :], op=add)
                # t3 = dinv * leaky_relu(t2) = max(dinv*t2, alpha*dinv*t2)
                t3s = wp.tile([P, D1], F32, tag="t3s")
                nc.scalar.activation(out=t3s[:nb, :], in_=t2[:nb, :],
                                     func=mybir.ActivationFunctionType.Copy,
                                     bias=0.0, scale=dvsb[:nb, b:b + 1])
                t3u = wp.tile([P, D1], F32, tag="t3u")
                nc.scalar.activation(out=t3u[:nb, :], in_=t3s[:nb, :],
                                     func=mybir.ActivationFunctionType.Copy,
                                     bias=0.0, scale=float(cfg.neg_slope))
                t3 = wp.tile([P, D1], F32, tag="t3")
                nc.vector.tensor_tensor(out=t3[:nb, :], in0=t3s[:nb, :],
                                        in1=t3u[:nb, :],
                                        op=mybir.AluOpType.max)
                tp = mmp.tile([P, P], F32, tag="mm")
                nc.tensor.transpose(out=tp[:, :nb], in_=t3[:nb, :],
                                    identity=ident[:nb, :nb])
                t4 = wp.tile([P, P], F32, tag="t4")
                nc.scalar.copy(out=t4[:, :nb], in_=tp[:, :nb])
                v = mmp.tile([P, D2], F32, tag="mm")
                nc.tensor.matmul(out=v[:nb, :], lhsT=t4[:, :nb],
                                 rhs=w2sb[:], start=True, stop=True)
                h2b = wp.tile([P, D2], F32, tag="h2b")
                nc.vector.tensor_copy(out=h2b[:nb, :], in_=v[:nb, :])
                nc.sync.dma_start(out=h2_loc[b * P:b * P + nb, :],
                                  in_=h2b[:nb, :])

            def l2_sink(b, nb, t0):
                t1 = wp.tile([P, D2], F32, tag="u1")
                nc.vector.tensor_scalar(out=t1[:nb, :], in0=t0[:nb, :],
                                        scalar1=dvsb[:nb, b:b + 1],
                                        scalar2=None, op0=mul)
                t2 = wp.tile([P, D2], F32, tag="u2")
                nc.vector.tensor_tensor(out=t2[:nb, :], in0=t1[:nb, :],
                                        in1=b2sb[:nb, :], op=add)
                nc.sync.dma_start(out=out_t[b * P:b * P + nb, :],
                                  in_=t2[:nb, :])

            agg_phase(1, h1_full, h1_loc, D1, l1_sink)

            nc.gpsimd.collective_compute(
                "AllGather", mybir.AluOpType.bypass,
                replica_groups=[list(range(cfg.ncores))],
                ins=[h2_loc.opt()], outs=[h2_full.opt()])

            agg_phase(2, h2_full, h2_loc, D2, l2_sink)

    nc.compile()
    return nc


# --------------------------------------------------------------------------
# entry point
# --------------------------------------------------------------------------

_CACHE = {}


def _run(cfg, inputs):
    in_maps, struct = prepare(cfg, inputs["x"], inputs["edge_index"],
                              inputs["W1"], inputs["b1"],
                              inputs["W2"], inputs["b2"])
    key = (cfg.n, cfg.e, struct["T"].tobytes())
    nc = _CACHE.get(key)
    if nc is None:
        nc = build_program(cfg, struct)
        _CACHE[key] = nc
    res = run_bass_kernel_spmd(nc, in_maps, list(range(cfg.ncores)))
    out = np.concatenate([res.results[c]["out_loc"]
                          for c in range(cfg.ncores)], axis=0)
    return out.astype(np.float32)


def kernel(x, edge_index, W1, b1, W2, b2):
    return _run(FULL, dict(x=x, edge_index=edge_index, W1=W1, b1=b1,
                           W2=W2, b2=b2))


# revision 14
# speedup vs baseline: 10.5533x; 10.5533x over previous
"""Two-layer GCN (GCNConv 256->128->64, leaky_relu) on 8 Trainium2 NeuronCores.

Strategy (graph/data parallel per the sharding hint):
  - Nodes are sharded contiguously across the 8 cores (12500 each); each core
    owns the destination rows of the scatter for its node range.
  - Symmetric normalization is folded into node features: with
    h~ = dinv ! (X @ W), the aggregation for node v is
    out[v] = dinv[v] * (sum_{e: dst=v} h~[src_e] + h~[v]) + b.
  - Each layer: dense transform on local nodes -> AllGather h~ (the halo
    exchange; with 8 cores and uniform random edges every core needs nearly
    every node) -> per 128-node dst block, dma_gather the source rows and
    segment-sum them with 0/1 selection-matrix matmuls accumulated in PSUM.
  - Edge tables (gather indices, dst slots) are precomputed host-side from
    edge_index; edges are sorted by (dst block group, src 32k-chunk, dst
    block) and padded to 128-edge tiles.  dma_gather indices are int16, which
    limits each gather to a 32768-row window of the source table - hence the
    4 source chunks.
"""
import math
import sys

import numpy as np

sys.path.insert(0, "/opt/trn_rl_repo")

import concourse.bacc as bacc  # noqa: E402
import concourse.mybir as mybir  # noqa: E402
import concourse.tile as tile  # noqa: E402
from concourse.bass_utils import run_bass_kernel_spmd  # noqa: E402

P = 128
F32 = mybir.dt.float32
I16 = mybir.dt.int16


class Cfg:
    def __init__(self, n, e, d0, d1, d2, ncores=8, grp=6, chunk=32768,
                 neg_slope=0.01):
        assert n % ncores == 0
        self.n, self.e = n, e
        self.d0, self.d1, self.d2 = d0, d1, d2
        self.ncores = ncores
        self.nloc = n // ncores
        self.nblk = math.ceil(self.nloc / P)
        self.grp = grp
        self.ngrp = math.ceil(self.nblk / grp)
        self.chunk = chunk
        self.nchunk = math.ceil(n / chunk)
        self.neg_slope = neg_slope

    def rows(self, b):
        return min(P, self.nloc - b * P)


FULL = Cfg(n=100000, e=1600000, d0=256, d1=128, d2=64)


# --------------------------------------------------------------------------
# host-side preprocessing
# --------------------------------------------------------------------------

def prepare(cfg, x, edge_index, W1, b1, W2, b2):
    """Shard + build per-core tables and the shared tile structure."""
    src = np.asarray(edge_index[0], dtype=np.int64)
    dst = np.asarray(edge_index[1], dtype=np.int64)

    deg = (np.bincount(dst, minlength=cfg.n) + 1).astype(np.float32)
    dinv = (np.float32(1.0) / np.sqrt(deg)).astype(np.float32)

    core = dst // cfg.nloc
    dstl = dst - core * cfg.nloc
    b = dstl >> 7
    k = src // cfg.chunk
    g = b // cfg.grp
    # stream order: core -> g -> k -> b (tiles within segment arbitrary)
    key = ((core * cfg.ngrp + g) * cfg.nchunk + k) * cfg.nblk + b
    order = np.argsort(key, kind="stable")
    src_s, b_s, k_s, core_s, dstl_s = (
        src[order], b[order], k[order], core[order], dstl[order])

    seg_id = ((core_s * cfg.nblk + b_s) * cfg.nchunk + k_s)
    counts = np.bincount(seg_id, minlength=cfg.ncores * cfg.nblk * cfg.nchunk)
    counts = counts.reshape(cfg.ncores, cfg.nblk, cfg.nchunk)
    # shared tile structure: max over cores
    T = np.ceil(counts.max(axis=0) / P).astype(np.int64)  # [nblk, nchunk]

    # segment stream offsets (in padded edge slots), shared across cores
    seg_off = np.zeros((cfg.nblk, cfg.nchunk), np.int64)
    blocks_of_g = [list(range(gg * cfg.grp, min((gg + 1) * cfg.grp, cfg.nblk)))
                   for gg in range(cfg.ngrp)]
    tot_tiles = 0
    for gg in range(cfg.ngrp):
        for kk in range(cfg.nchunk):
            for bb in blocks_of_g[gg]:
                seg_off[bb, kk] = tot_tiles * P
                tot_tiles += int(T[bb, kk])
    etot = tot_tiles * P

    # per-core padded arrays
    xs = np.asarray(x, dtype=np.float32)
    W1 = np.asarray(W1, np.float32)
    W2 = np.asarray(W2, np.float32)
    b1 = np.asarray(b1, np.float32)
    b2 = np.asarray(b2, np.float32)
    b1bc = np.tile(b1[None, :], (P, 1)).astype(np.float32)
    b2bc = np.tile(b2[None, :], (P, 1)).astype(np.float32)

    in_maps = []
    for c in range(cfg.ncores):
        m = core_s == c
        src_c, b_c, k_c, dstl_c = src_s[m], b_s[m], k_s[m], dstl_s[m]
        seg_c = (b_c * cfg.nchunk + k_c)
        # rank within segment: edges are segment-contiguous in stream order
        ne = len(src_c)
        if ne:
            newseg = np.r_[True, seg_c[1:] != seg_c[:-1]]
            seg_start = np.maximum.accumulate(
                np.where(newseg, np.arange(ne), 0))
            pos_in_seg = np.arange(ne) - seg_start
        else:
            pos_in_seg = np.zeros(0, np.int64)
        padded_pos = seg_off[b_c, k_c] + pos_in_seg

        idx_pad = np.zeros(etot, np.int16)
        idx_pad[padded_pos] = (src_c - k_c * cfg.chunk).astype(np.int16)
        dr_pad = np.full(etot, -1.0, np.float32)
        dr_pad[padded_pos] = (dstl_c - b_c * P).astype(np.float32)

        # idx table: wrapped [16, etot//16] (entry [i%16, i//16]), x8 down 128
        idx16 = idx_pad.reshape(etot // 16, 16).T.copy()
        idx_tab = np.tile(idx16, (8, 1))
        # dstrel table: [128, tot_tiles], [p, t] = edge t*128+p
        dstrel_tab = dr_pad.reshape(tot_tiles, P).T.copy()

        lo = c * cfg.nloc
        xT = np.ascontiguousarray(xs[lo:lo + cfg.nloc].T)
        dloc = dinv[lo:lo + cfg.nloc]
        dcols = np.ones(cfg.nblk * P, np.float32)
        dcols[:cfg.nloc] = dloc
        dinv_cols = dcols.reshape(cfg.nblk, P).T.copy()

        in_maps.append({
            "xT": xT, "dinv_cols": dinv_cols,
            "W1": W1, "W2": W2, "b1bc": b1bc, "b2bc": b2bc,
            "idx_tab": idx_tab, "dstrel_tab": dstrel_tab,
        })

    struct = dict(T=T, blocks_of_g=blocks_of_g, tot_tiles=tot_tiles)
    return in_maps, struct


# --------------------------------------------------------------------------
# device program
# --------------------------------------------------------------------------

def build_program(cfg, struct, repeat=1):
    T = struct["T"]
    blocks_of_g = struct["blocks_of_g"]
    tot_tiles = struct["tot_tiles"]
    NB, NK, NG = cfg.nblk, cfg.nchunk, cfg.ngrp
    D0, D1, D2 = cfg.d0, cfg.d1, cfg.d2
    K0 = D0 // P
    G = cfg.grp

    # per-group layout in the tile stream
    grp_tiles = []           # tiles per group
    gk_info = []             # [g][k] -> (col0 within group, ntiles)
    blk_tile_cols = {}       # b -> [(col within group, ntiles, k)]
    grp_col0 = []            # group offset in global tile stream
    tot = 0
    for g in range(NG):
        grp_col0.append(tot)
        col = 0
        ks = []
        for k in range(NK):
            c0 = col
            for b in blocks_of_g[g]:
                t = int(T[b, k])
                if t:
                    blk_tile_cols.setdefault(b, []).append((col, t, k))
                    col += t
            ks.append((c0, col - c0))
        gk_info.append(ks)
        grp_tiles.append(col)
        tot += col
    assert tot == tot_tiles
    TG = max(grp_tiles)
    TBK = max(int(T.max()), 1)

    nc = bacc.Bacc("TRN2", target_bir_lowering=False, debug=False,
                   num_devices=cfg.ncores)
    xT = nc.dram_tensor("xT", [D0, cfg.nloc], F32, kind="ExternalInput")
    dinv_t = nc.dram_tensor("dinv_cols", [P, NB], F32, kind="ExternalInput")
    W1 = nc.dram_tensor("W1", [D0, D1], F32, kind="ExternalInput")
    W2 = nc.dram_tensor("W2", [D1, D2], F32, kind="ExternalInput")
    b1t = nc.dram_tensor("b1bc", [P, D1], F32, kind="ExternalInput")
    b2t = nc.dram_tensor("b2bc", [P, D2], F32, kind="ExternalInput")
    idx_t = nc.dram_tensor("idx_tab", [P, tot_tiles * 8], I16,
                           kind="ExternalInput")
    drel_t = nc.dram_tensor("dstrel_tab", [P, tot_tiles], F32,
                            kind="ExternalInput")
    out_t = nc.dram_tensor("out_loc", [cfg.nloc, D2], F32,
                           kind="ExternalOutput")

    eq = mybir.AluOpType.is_equal
    mul = mybir.AluOpType.mult
    add = mybir.AluOpType.add
    lrelu = mybir.ActivationFunctionType.Lrelu

    with tile.TileContext(nc) as tc:
        with (
            tc.tile_pool(name="const", bufs=1) as cp,
            tc.tile_pool(name="work", bufs=2) as wp,
            tc.tile_pool(name="sm", bufs=3) as smp,
            tc.tile_pool(name="mm", bufs=4, space="PSUM") as mmp,
            tc.tile_pool(name="agg", bufs=2, space="PSUM") as aggp,
            tc.tile_pool(name="dram", bufs=1, space="DRAM") as drp,
        ):
            h1_loc = drp.tile([cfg.nloc, D1], F32, tag="h1_loc")
            h2_loc = drp.tile([cfg.nloc, D2], F32, tag="h2_loc")

            w1sb = cp.tile([P, K0, D1], F32)
            for k0 in range(K0):
                nc.sync.dma_start(out=w1sb[:, k0, :],
                                  in_=W1[k0 * P:(k0 + 1) * P, :])
            w2sb = cp.tile([P, D2], F32)
            nc.sync.dma_start(out=w2sb[:], in_=W2[:])
            b1sb = cp.tile([P, D1], F32)
            nc.sync.dma_start(out=b1sb[:], in_=b1t[:])
            b2sb = cp.tile([P, D2], F32)
            nc.sync.dma_start(out=b2sb[:], in_=b2t[:])
            dvsb = cp.tile([P, NB], F32)
            nc.sync.dma_start(out=dvsb[:], in_=dinv_t[:])
            iota = cp.tile([P, P], F32)
            nc.gpsimd.iota(iota[:], pattern=[[1, P]], base=0,
                           channel_multiplier=0,
                           allow_small_or_imprecise_dtypes=True)
            ident = cp.tile([P, P], F32)
            from concourse.masks import make_identity
            make_identity(nc, ident[:])

            # ---------------- dense layer 1: h1_loc = dinv * (x @ W1)
            def dense1():
                for b in range(NB):
                    nb = cfg.rows(b)
                    xt = wp.tile([P, K0, P], F32, tag="xt")
                    for k0 in range(K0):
                        nc.sync.dma_start(
                            out=xt[:, k0, :nb],
                            in_=xT[k0 * P:(k0 + 1) * P, b * P:b * P + nb])
                    ps = mmp.tile([P, D1], F32, tag="mm")
                    for k0 in range(K0):
                        nc.tensor.matmul(out=ps[:nb, :], lhsT=xt[:, k0, :nb],
                                         rhs=w1sb[:, k0, :],
                                         start=(k0 == 0), stop=(k0 == K0 - 1))
                    h1w = wp.tile([P, D1], F32, tag="h1w")
                    nc.vector.tensor_scalar(out=h1w[:nb, :], in0=ps[:nb, :],
                                            scalar1=dvsb[:nb, b:b + 1],
                                            scalar2=None, op0=mul)
                    nc.sync.dma_start(out=h1_loc[b * P:b * P + nb, :],
                                      in_=h1w[:nb, :])

            # ---------------- layer-1 aggregation + fused layer-2 dense
            def agg_phase(layer, h_full, h_self, D, out_sink):
                for g in range(NG):
                    Tg = grp_tiles[g]
                    if Tg == 0:
                        continue
                    gt0 = grp_col0[g]
                    stage_raw = wp.tile([P, TG * D1], F32, tag="stage")
                    stage = stage_raw[:].rearrange("p (t d) -> p t d", d=D)
                    idxt = wp.tile([P, TG * 8], I16, tag="idx")
                    drt = wp.tile([P, TG], F32, tag="drel")
                    nc.sync.dma_start(out=idxt[:, :Tg * 8],
                                      in_=idx_t[:, gt0 * 8:(gt0 + Tg) * 8])
                    nc.sync.dma_start(out=drt[:, :Tg],
                                      in_=drel_t[:, gt0:gt0 + Tg])
                    for k in range(NK):
                        c0, ntk = gk_info[g][k]
                        if ntk == 0:
                            continue
                        hi = min((k + 1) * cfg.chunk, cfg.n)
                        nc.gpsimd.dma_gather(
                            out_ap=stage[:, c0:c0 + ntk, :],
                            in_ap=h_full[k * cfg.chunk:hi, :],
                            idxs_ap=idxt[:, c0 * 8:(c0 + ntk) * 8],
                            num_idxs=ntk * P, num_idxs_reg=ntk * P,
                            elem_size=D, single_packet=False)
                    agg = aggp.tile([P, G * D], F32, tag="agg")
                    for slot, b in enumerate(blocks_of_g[g]):
                        segs = blk_tile_cols.get(b, [])
                        ntot = sum(t for _, t, _ in segs)
                        nb = cfg.rows(b)
                        left = ntot
                        first = True
                        for (col, t, k) in segs:
                            sm = smp.tile([P, TBK, P], F32, tag="smulti")
                            nc.vector.tensor_tensor(
                                out=sm[:, :t, :],
                                in0=iota[:].unsqueeze(1)
                                    .broadcast_to([P, t, P]),
                                in1=drt[:, col:col + t].unsqueeze(2)
                                    .broadcast_to([P, t, P]),
                                op=eq)
                            for tt in range(t):
                                nc.tensor.matmul(
                                    out=agg[:, slot * D:(slot + 1) * D],
                                    lhsT=sm[:, tt, :],
                                    rhs=stage[:, col + tt, :],
                                    start=first, stop=(left == 1),
                                    skip_group_check=True)
                                first = False
                                left -= 1
                        # flush
                        selfb = wp.tile([P, D], F32, tag=f"selfb{layer}")
                        nc.sync.dma_start(
                            out=selfb[:nb, :],
                            in_=h_self[b * P:b * P + nb, :])
                        t0 = wp.tile([P, D], F32, tag=f"t0_{layer}")
                        if ntot:
                            nc.vector.tensor_tensor(
                                out=t0[:nb, :],
                                in0=agg[:nb, slot * D:(slot + 1) * D],
                                in1=selfb[:nb, :], op=add)
                        else:
                            nc.vector.tensor_copy(out=t0[:nb, :],
                                                  in_=selfb[:nb, :])
                        out_sink(b, nb, t0)

            def l1_sink(b, nb, t0):
                t1 = wp.tile([P, D1], F32, tag="t1")
                nc.vector.tensor_scalar(out=t1[:nb, :], in0=t0[:nb, :],
                                        scalar1=dvsb[:nb, b:b + 1],
                                        scalar2=None, op0=mul)
                t2 = wp.tile([P, D1], F32, tag="t2")
                nc.vector.tensor_tensor(out=t2[:nb, :], in0=t1[:nb, :],
                                        in1=b1sb[:nb, :], op=add)
                # t3 = dinv * leaky_relu(t2) = max(dinv*t2, alpha*dinv*t2)
                t3s = wp.tile([P, D1], F32, tag="t3s")
                nc.scalar.activation(out=t3s[:nb, :], in_=t2[:nb, :],
                                     func=mybir.ActivationFunctionType.Copy,
                                     bias=0.0, scale=dvsb[:nb, b:b + 1])
                t3u = wp.tile([P, D1], F32, tag="t3u")
                nc.scalar.activation(out=t3u[:nb, :], in_=t3s[:nb, :],
                                     func=mybir.ActivationFunctionType.Copy,
                                     bias=0.0, scale=float(cfg.neg_slope))
                t3 = wp.tile([P, D1], F32, tag="t3")
                nc.vector.tensor_tensor(out=t3[:nb, :], in0=t3s[:nb, :],
                                        in1=t3u[:nb, :],
                                        op=mybir.AluOpType.max)
                tp = mmp.tile([P, P], F32, tag="mm")
                nc.tensor.transpose(out=tp[:, :nb], in_=t3[:nb, :],
                                    identity=ident[:nb, :nb])
                t4 = wp.tile([P, P], F32, tag="t4")
                nc.scalar.copy(out=t4[:, :nb], in_=tp[:, :nb])
                v = mmp.tile([P, D2], F32, tag="mm")
                nc.tensor.matmul(out=v[:nb, :], lhsT=t4[:, :nb],
                                 rhs=w2sb[:], start=True, stop=True)
                h2b = wp.tile([P, D2], F32, tag="h2b")
                nc.vector.tensor_copy(out=h2b[:nb, :], in_=v[:nb, :])
                nc.sync.dma_start(out=h2_loc[b * P:b * P + nb, :],
                                  in_=h2b[:nb, :])

            def l2_sink(b, nb, t0):
                t1 = wp.tile([P, D2], F32, tag="u1")
                nc.vector.tensor_scalar(out=t1[:nb, :], in0=t0[:nb, :],
                                        scalar1=dvsb[:nb, b:b + 1],
                                        scalar2=None, op0=mul)
                t2 = wp.tile([P, D2], F32, tag="u2")
                nc.vector.tensor_tensor(out=t2[:nb, :], in0=t1[:nb, :],
                                        in1=b2sb[:nb, :], op=add)
                nc.sync.dma_start(out=out_t[b * P:b * P + nb, :],
                                  in_=t2[:nb, :])

            mode = getattr(cfg, "repeat_mode", "all")
            h1_full = h2_full = None
            for _rep in range(repeat):
                rep_all = mode == "all" or _rep == 0
                if rep_all or mode == "collectives":
                    h1_full = drp.tile([cfg.n, D1], F32,
                                       tag=f"h1_full{_rep}",
                                       addr_space="Shared",
                                       name=f"h1_full{_rep}")
                    h2_full = drp.tile([cfg.n, D2], F32,
                                       tag=f"h2_full{_rep}",
                                       addr_space="Shared",
                                       name=f"h2_full{_rep}")
                if rep_all:
                    dense1()
                if rep_all or mode == "collectives":
                    nc.gpsimd.collective_compute(
                        "AllGather", mybir.AluOpType.bypass,
                        replica_groups=[list(range(cfg.ncores))],
                        ins=[h1_loc.opt()], outs=[h1_full.opt()])
                if rep_all or mode == "agg":
                    agg_phase(1, h1_full, h1_loc, D1, l1_sink)
                if rep_all or mode == "collectives":
                    nc.gpsimd.collective_compute(
                        "AllGather", mybir.AluOpType.bypass,
                        replica_groups=[list(range(cfg.ncores))],
                        ins=[h2_loc.opt()], outs=[h2_full.opt()])
                if rep_all or mode == "agg":
                    agg_phase(2, h2_full, h2_loc, D2, l2_sink)

    nc.compile()
    return nc


# --------------------------------------------------------------------------
# entry point
# --------------------------------------------------------------------------

_CACHE = {}


def _run(cfg, inputs):
    in_maps, struct = prepare(cfg, inputs["x"], inputs["edge_index"],
                              inputs["W1"], inputs["b1"],
                              inputs["W2"], inputs["b2"])
    key = (cfg.n, cfg.e, struct["T"].tobytes())
    nc = _CACHE.get(key)
    if nc is None:
        nc = build_program(cfg, struct)
        _CACHE[key] = nc
    res = run_bass_kernel_spmd(nc, in_maps, list(range(cfg.ncores)))
    out = np.concatenate([res.results[c]["out_loc"]
                          for c in range(cfg.ncores)], axis=0)
    return out.astype(np.float32)


def kernel(x, edge_index, W1, b1, W2, b2):
    return _run(FULL, dict(x=x, edge_index=edge_index, W1=W1, b1=b1,
                           W2=W2, b2=b2))
